# revision 18
# baseline (speedup 1.0000x reference)
"""Dense Synthesizer Attention — Trainium2 Bass kernel.

Sharding: data-parallel over batch. B=8 batch elements, 8 NeuronCores,
one batch element per core, zero collectives.

Per-core computation (S=1024 tokens, F=512 feat, H=8 heads, dk=64), all
matmuls bf16 with fp32 PSUM accumulation; q/value transposed and weights
cast to bf16 on the host:
    hT  = relu(w1^T @ qT + b1)          [1024, 1024]
    awT = w2^T @ hT + b2                [512, 1024]
    heads processed in PAIRS (2f, 2f+1) that share awT chunk f:
      scores: two K=64 matmuls run CONCURRENTLY in PE row-groups
        (0,0)/(64,0) via tile_position — 2x effective throughput
      E = exp(scores/8) bf16 (ACT, no accumulator); row sums via DVE
        reduce_sum (4x packed bf16 mode)
      attn_v: yT pair = v^T @ E with the two heads col-split (0,0)/(0,64)
        into one [128,512] PSUM tile — again 2x
      rinv = 1/rsum is transposed (PE) + flattened (DMA) + broadcast
        (GPSIMD partition_broadcast) to [128,S], and yT is scaled by it
        during the PSUM->SBUF copy (DVE tensor_tensor mult)
    out[m] = sum_pairs (yT_scaled^T @ wo) accumulated IN PSUM (K=128 per
        pair step), single DVE bias add per m-chunk, immediate DMA out.

The softmax exp stream on the Scalar engine (64 x [128,1024] ACTIVATEs)
is the pacing resource; the PE pair loop, DVE reductions/copies, and
GPSIMD broadcasts all fit inside its envelope. Input DMAs are split into
per-chunk transfers across the sync/gpsimd/vector queues so mlp1 starts
~2us in (HAM warms early); constants land staggered by first use.
"""

import math

import numpy as np

B, S, F = 8, 1024, 512
H, DK = 8, 64
HID = 2 * F
P = 128
NPAIR = H // 2

N_CORES = 8

DEBUG = False

_CACHED_NC = None


def _build_nc(repeat=1):
    from contextlib import ExitStack

    import concourse.mybir as mybir
    import concourse.tile as tile
    from concourse import bacc

    dt = mybir.dt
    f32 = dt.float32
    bf16 = dt.bfloat16

    SC = S // P      # 8 token chunks
    FC = F // P      # 4 feature chunks
    KC = HID // P    # 8 hidden chunks
    NS = S // 512    # 2 moving-dim slices

    nc = bacc.Bacc(
        "TRN2",
        target_bir_lowering=False,
        debug=False,
        num_devices=N_CORES,
    )

    q_d = nc.declare_dram_parameter("qT", [F, S], bf16, isOutput=False)
    v_d = nc.declare_dram_parameter("vT", [F, S], bf16, isOutput=False)
    w1_d = nc.declare_dram_parameter("w1", [F, HID], bf16, isOutput=False)
    w2_d = nc.declare_dram_parameter("w2", [HID, F], bf16, isOutput=False)
    wv_d = nc.declare_dram_parameter("wv", [F, F], bf16, isOutput=False)
    wo_d = nc.declare_dram_parameter("wo", [F, F], bf16, isOutput=False)
    b1_d = nc.declare_dram_parameter("b1r", [P, KC], f32, isOutput=False)
    b2_d = nc.declare_dram_parameter("b2r", [P, FC], f32, isOutput=False)
    bv_d = nc.declare_dram_parameter("bvb", [P, F], f32, isOutput=False)
    bo_d = nc.declare_dram_parameter("bob", [P, F], f32, isOutput=False)
    id_d = nc.declare_dram_parameter("ident", [P, P], f32, isOutput=False)
    out_d = nc.declare_dram_parameter("out", [S, F], f32, isOutput=True)
    dbg = None
    if DEBUG:
        dbg = dict(
            hT=nc.declare_dram_parameter("dbg_hT", [P, KC * S], bf16,
                                         isOutput=True),
            aw=nc.declare_dram_parameter("dbg_aw", [P, FC * S], bf16,
                                         isOutput=True),
            v=nc.declare_dram_parameter("dbg_v", [P, SC * F], bf16,
                                        isOutput=True),
            eA0=nc.declare_dram_parameter("dbg_eA0", [P, SC * S], bf16,
                                          isOutput=True),
            eB0=nc.declare_dram_parameter("dbg_eB0", [P, SC * S], bf16,
                                          isOutput=True),
            rs=nc.declare_dram_parameter("dbg_rs", [P, H * SC], f32,
                                         isOutput=True),
            ri=nc.declare_dram_parameter("dbg_ri", [P, H * SC], f32,
                                         isOutput=True),
            rb=nc.declare_dram_parameter("dbg_rb", [P, S], f32,
                                         isOutput=True),
            yT=nc.declare_dram_parameter("dbg_yT", [P, FC * S], bf16,
                                         isOutput=True),
        )

    with ExitStack() as ctx:
        tc = ctx.enter_context(tile.TileContext(nc))

        const = ctx.enter_context(tc.tile_pool(name="const", bufs=1))
        big = ctx.enter_context(tc.tile_pool(name="big", bufs=1))
        sh16 = ctx.enter_context(tc.tile_pool(name="sh16", bufs=3))
        rpool = ctx.enter_context(tc.tile_pool(name="rpool", bufs=1))
        opool = ctx.enter_context(tc.tile_pool(name="opool", bufs=1))
        flip = ctx.enter_context(tc.tile_pool(name="flip", bufs=1))

        ps_sc = ctx.enter_context(tc.tile_pool(name="ps_sc", bufs=2, space="PSUM"))
        ps_yt = ctx.enter_context(tc.tile_pool(name="ps_yt", bufs=2, space="PSUM"))
        ps512 = ctx.enter_context(tc.tile_pool(name="ps512", bufs=2, space="PSUM"))

        # ---- constants + inputs: fine-grained DMAs, 3 queues, staged by
        # first use so mlp1's first matmul can start ~2us in ----
        w1sb = const.tile([P, FC, HID], bf16)
        w2sb = const.tile([P, KC, F], bf16)
        wvsb = const.tile([P, FC, F], bf16)
        wosb = const.tile([P, FC, F], bf16)
        b1sb = const.tile([P, KC], f32)
        b2sb = const.tile([P, FC], f32)
        bvsb = const.tile([P, F], f32)
        bosb = const.tile([P, F], f32)
        idsb = const.tile([P, P], f32)

        qTsb = big.tile([P, FC, S], bf16, tag="qx")
        valTsb = sh16.tile([P, FC, S], bf16, tag="sh")

        qr = q_d.rearrange("(c p) s -> p c s", p=P)
        w1r = w1_d.rearrange("(c p) k -> p c k", p=P)

        nc.sync.dma_start(b1sb, b1_d[:, :])
        for c in range(FC):
            nc.sync.dma_start(w1sb[:, c, :], w1r[:, c, :])
            nc.gpsimd.dma_start(qTsb[:, c, :512], qr[:, c, :512])
        nc.gpsimd.dma_start(qTsb[:, :, 512:], qr[:, :, 512:])
        nc.scalar.dma_start(w2sb, w2_d.rearrange("(c p) f -> p c f", p=P))
        nc.sync.dma_start(valTsb, v_d.rearrange("(c p) s -> p c s", p=P))
        nc.gpsimd.dma_start(wvsb, wv_d.rearrange("(c p) f -> p c f", p=P))
        nc.gpsimd.dma_start(b2sb, b2_d[:, :])
        nc.scalar.dma_start(wosb, wo_d.rearrange("(c p) f -> p c f", p=P))
        nc.sync.dma_start(bvsb, bv_d[:, :])
        nc.sync.dma_start(bosb, bo_d[:, :])
        nc.gpsimd.dma_start(idsb, id_d[:, :])

        consts = (w1sb, w2sb, wvsb, wosb, b1sb, b2sb, bvsb, bosb, idsb)
        for _rep in range(repeat):
            _build_body(nc, mybir, big, sh16, rpool, opool, flip,
                        ps_sc, ps_yt, ps512, qTsb, valTsb, out_d, consts,
                        dbg)

    nc.compile()
    return nc


def _build_body(nc, mybir, big, sh16, rpool, opool, flip,
                ps_sc, ps_yt, ps512, qTsb, valTsb, out_d, consts, dbg=None):
    w1sb, w2sb, wvsb, wosb, b1sb, b2sb, bvsb, bosb, idsb = consts
    dt = mybir.dt
    AF = mybir.ActivationFunctionType
    ALU = mybir.AluOpType
    AX = mybir.AxisListType
    f32, bf16 = dt.float32, dt.bfloat16
    SC, FC, KC, NS = S // P, F // P, HID // P, S // 512

    # ---- mlp1: hT = relu(w1^T @ qT + b1)  [HID, S] ----
    hTsb = big.tile([P, KC, S], bf16, tag="hT")
    for n in range(NS):
        for m in range(KC):
            h_p = ps512.tile([P, 512], f32, tag="ps")
            for c in range(FC):
                nc.tensor.matmul(
                    h_p,
                    w1sb[:, c, m * P:(m + 1) * P],
                    qTsb[:, c, n * 512:(n + 1) * 512],
                    start=(c == 0),
                    stop=(c == FC - 1),
                )
            nc.vector.tensor_scalar(
                hTsb[:, m, n * 512:(n + 1) * 512], h_p,
                b1sb[:, m:m + 1], 0.0, ALU.add, ALU.max,
            )

    # ---- mlp2 (chunk f feeds head pair f) ----
    awTsb = big.tile([P, FC, S], bf16, tag="awT")

    def mlp2_items(m):
        items = []
        state = {}

        def mk(n, c):
            def go():
                if c == 0:
                    state[n] = ps512.tile([P, 512], f32, tag="ps", name="m2p")
                nc.tensor.matmul(
                    state[n],
                    w2sb[:, c, m * P:(m + 1) * P],
                    hTsb[:, c, n * 512:(n + 1) * 512],
                    start=(c == 0),
                    stop=(c == KC - 1),
                )
                if c == KC - 1:
                    nc.vector.tensor_scalar_add(
                        awTsb[:, m, n * 512:(n + 1) * 512], state[n],
                        b2sb[:, m:m + 1],
                    )
            return go

        for n in range(NS):
            for c in range(KC):
                items.append(mk(n, c))
        return items

    # ---- v projection ----
    vsb = big.tile([P, SC, F], bf16, tag="v")

    def vproj_items(m):
        state = {}

        def mk(c):
            def go():
                if c == 0:
                    state[0] = ps512.tile([P, 512], f32, tag="ps", name="vpp")
                nc.tensor.matmul(
                    state[0],
                    valTsb[:, c, m * P:(m + 1) * P],
                    wvsb[:, c, :],
                    start=(c == 0),
                    stop=(c == FC - 1),
                )
                if c == FC - 1:
                    nc.vector.tensor_add(vsb[:, m, :], state[0], bvsb)
            return go

        return [mk(c) for c in range(FC)]

    # ---- per-pair state ----
    yTsb = big.tile([P, FC, S], bf16, tag="qx")  # reuses qT slot
    scale = 1.0 / math.sqrt(DK)
    rsum_all = rpool.tile([P, H, SC], f32, tag="rs")
    rinv_all = rpool.tile([P, H, SC], f32, tag="ri")
    o_all = opool.tile([P, SC, F], f32, tag="o")


    def pair_slot(pr, filler):
        """Head pair (2pr, 2pr+1): concurrent row-split scores, exp on ACT,
        DVE row sums, col-split attn_v interleaved at 1-chunk lag, then the
        rinv flip (PE transpose -> flatten DMA -> partition_broadcast) and
        the rinv-scaled yT copies. `filler` items (~1 matmul each) pad the
        PE stream while ACT chews the exps."""
        hA, hB = 2 * pr, 2 * pr + 1
        awp = awTsb[:, pr, :]
        eA = sh16.tile([P, SC, S], bf16, tag="sh")
        eB = sh16.tile([P, SC, S], bf16, tag="sh")

        yt_state = {}

        def attn_items(c):
            def mk(n, half, ldwf):
                def go():
                    if c == 0 and half == 0:
                        yt_state[n] = ps_yt.tile([P, 512], f32, tag="pt",
                                                 name="ytp")
                    e = eA if half == 0 else eB
                    h = hA if half == 0 else hB
                    out = (yt_state[n][0:DK, :] if half == 0
                           else yt_state[n][DK:P, :])
                    mm = nc.tensor.matmul(
                        out,
                        vsb[:, c, h * DK:(h + 1) * DK],
                        e[:, c, n * 512:(n + 1) * 512],
                        start=(c == 0),
                        stop=(c == SC - 1),
                        skip_group_check=True,
                    )
                    if ldwf:
                        mm.ins.ldweights = False
                return go

            return [mk(0, 0, False), mk(0, 1, False),
                    mk(1, 0, True), mk(1, 1, True)]

        per_slot = (len(filler) + SC - 1) // SC if filler else 0
        fi = 0
        for m in range(SC):
            pA = ps_sc.tile([P, S], f32, tag="sc")
            pB = ps_sc.tile([P, S], f32, tag="sc")
            for n in range(NS):
                mmA = nc.tensor.matmul(
                    pA[:, n * 512:(n + 1) * 512],
                    awp[0:DK, m * P:(m + 1) * P],
                    awp[0:DK, n * 512:(n + 1) * 512],
                    start=True, stop=True,
                )
                mmB = nc.tensor.matmul(
                    pB[:, n * 512:(n + 1) * 512],
                    awp[DK:P, m * P:(m + 1) * P],
                    awp[DK:P, n * 512:(n + 1) * 512],
                    start=True, stop=True,
                )
                if n == 1:
                    mmA.ins.ldweights = False
                    mmB.ins.ldweights = False
            nc.scalar.activation(eA[:, m, :], pA, AF.Exp, scale=scale)
            nc.vector.reduce_sum(rsum_all[:, hA, m:m + 1], eA[:, m, :], AX.X)
            nc.scalar.activation(eB[:, m, :], pB, AF.Exp, scale=scale)
            nc.vector.reduce_sum(rsum_all[:, hB, m:m + 1], eB[:, m, :], AX.X)
            if m >= 1:
                for go in attn_items(m - 1):
                    go()
            for _ in range(per_slot):
                if fi < len(filler):
                    filler[fi]()
                    fi += 1
        while fi < len(filler):
            filler[fi]()
            fi += 1
        for go in attn_items(SC - 1):
            go()

        nc.vector.reciprocal(rinv_all[:, hA:hA + 2, :],
                             rsum_all[:, hA:hA + 2, :])
        for n in range(NS):
            nc.vector.tensor_copy(
                yTsb[:, pr, n * 512:(n + 1) * 512], yt_state[n],
            )
        return eA, eB

    # ---- final: row-split pair matmuls (0,0)/(64,0), softmax division
    # fused into the per-head scalar_tensor_tensor accumulation ----
    def final_pair_items(pr):
        hA, hB = 2 * pr, 2 * pr + 1

        def mk(m):
            def go():
                oA = ps512.tile([P, 512], f32, tag="ps", name="fpA")
                nc.tensor.matmul(
                    oA, yTsb[0:DK, pr, m * P:(m + 1) * P],
                    wosb[0:DK, pr, :], start=True, stop=True,
                )
                oB = ps512.tile([P, 512], f32, tag="ps", name="fpB")
                nc.tensor.matmul(
                    oB, yTsb[DK:P, pr, m * P:(m + 1) * P],
                    wosb[DK:P, pr, :], start=True, stop=True,
                )
                nc.vector.scalar_tensor_tensor(
                    o_all[:, m, :], oA, rinv_all[:, hA, m:m + 1],
                    bosb if pr == 0 else o_all[:, m, :],
                    ALU.mult, ALU.add,
                )
                nc.vector.scalar_tensor_tensor(
                    o_all[:, m, :], oB, rinv_all[:, hB, m:m + 1],
                    o_all[:, m, :], ALU.mult, ALU.add,
                )
                if pr == NPAIR - 1:
                    nc.sync.dma_start(out_d[m * P:(m + 1) * P, :],
                                      o_all[:, m, :])
            return go

        return [mk(m) for m in range(SC)]

    # ---- software pipeline ----
    for go in mlp2_items(0):
        go()
    slot_fill = [
        (vproj_items(0) + vproj_items(1) + vproj_items(2) + vproj_items(3)
         + vproj_items(4) + vproj_items(5) + vproj_items(6) + vproj_items(7)
         + mlp2_items(1)),
        mlp2_items(2) + mlp2_items(3) + final_pair_items(0),
        final_pair_items(1),
        final_pair_items(2),
    ]
    e_keep = {}
    for pr in range(NPAIR):
        e_keep["eA"], e_keep["eB"] = pair_slot(pr, slot_fill[pr])
        if dbg is not None and pr == 0:
            nc.sync.dma_start(dbg["eA0"].rearrange("p (c s) -> p c s", c=SC),
                              e_keep["eA"])
            nc.sync.dma_start(dbg["eB0"].rearrange("p (c s) -> p c s", c=SC),
                              e_keep["eB"])
    if dbg is not None:
        nc.sync.dma_start(dbg["hT"].rearrange("p (c s) -> p c s", c=KC), hTsb)
        nc.sync.dma_start(dbg["aw"].rearrange("p (c s) -> p c s", c=FC),
                          awTsb)
        nc.sync.dma_start(dbg["v"].rearrange("p (c s) -> p c s", c=SC), vsb)
        nc.sync.dma_start(dbg["rs"].rearrange("p (h c) -> p h c", h=H),
                          rsum_all)
        nc.sync.dma_start(dbg["ri"].rearrange("p (h c) -> p h c", h=H),
                          rinv_all)
        nc.sync.dma_start(dbg["yT"].rearrange("p (c s) -> p c s", c=FC),
                          yTsb)

    # ---- last pair's final runs as the epilogue ----
    for go in final_pair_items(NPAIR - 1):
        go()


def _get_nc(repeat=1):
    global _CACHED_NC
    if _CACHED_NC is None:
        _CACHED_NC = _build_nc(repeat)
    return _CACHED_NC


def _make_in_maps(inputs):
    query = np.asarray(inputs["query"], np.float32)
    value = np.asarray(inputs["value"], np.float32)
    import ml_dtypes
    bf = ml_dtypes.bfloat16
    w1 = np.asarray(inputs["w1"], np.float32)
    b1 = np.asarray(inputs["b1"], np.float32)
    w2 = np.asarray(inputs["w2"], np.float32)
    b2 = np.asarray(inputs["b2"], np.float32)
    wv = np.asarray(inputs["wv"], np.float32)
    bv = np.asarray(inputs["bv"], np.float32)
    wo = np.asarray(inputs["wo"], np.float32)
    bo = np.asarray(inputs["bo"], np.float32)

    b1r = np.ascontiguousarray(b1.reshape(HID // P, P).T)
    b2r = np.ascontiguousarray(b2.reshape(F // P, P).T)
    bvb = np.ascontiguousarray(np.broadcast_to(bv, (P, F)))
    bob = np.ascontiguousarray(np.broadcast_to(bo, (P, F)))

    shared = dict(w1=w1.astype(bf), w2=w2.astype(bf), wv=wv.astype(bf),
                  wo=wo.astype(bf), b1r=b1r, b2r=b2r, bvb=bvb, bob=bob,
                  ident=np.eye(P, dtype=np.float32))
    return [dict(qT=np.ascontiguousarray(query[i].T).astype(bf),
                 vT=np.ascontiguousarray(value[i].T).astype(bf), **shared)
            for i in range(N_CORES)]


def kernel(**inputs):
    in_maps = _make_in_maps(inputs)

    from concourse.bass_utils import run_bass_kernel_spmd

    nc = _get_nc()
    res = run_bass_kernel_spmd(nc, in_maps, core_ids=list(range(N_CORES)))
    out = np.stack([res.results[i]["out"] for i in range(N_CORES)], axis=0)
    return out.astype(np.float32)


if __name__ == "__main__":
    nc = _get_nc()
    print("built ok")


# revision 19
# speedup vs baseline: 1.2831x; 1.2831x over previous
"""Dense Synthesizer Attention — Trainium2 Bass kernel.

Sharding: data-parallel over batch. B=8 batch elements, 8 NeuronCores,
one batch element per core, zero collectives.

Per-core computation (S=1024 tokens, F=512 feat, H=8 heads, dk=64), all
matmuls bf16 with fp32 PSUM accumulation; q/value transposed and weights
cast to bf16 on the host:
    hT  = relu(w1^T @ qT + b1)          [1024, 1024]
    awT = w2^T @ hT + b2                [512, 1024]
    heads processed in PAIRS (2f, 2f+1) that share awT chunk f:
      scores: two K=64 matmuls run CONCURRENTLY in PE row-groups
        (0,0)/(64,0) via tile_position — 2x effective throughput
      E = exp(scores/8) bf16 (ACT, no accumulator); row sums via DVE
        reduce_sum (4x packed bf16 mode)
      attn_v: yT pair = v^T @ E with the two heads col-split (0,0)/(0,64)
        into one [128,512] PSUM tile — again 2x
      rinv = 1/rsum is transposed (PE) + flattened (DMA) + broadcast
        (GPSIMD partition_broadcast) to [128,S], and yT is scaled by it
        during the PSUM->SBUF copy (DVE tensor_tensor mult)
    out[m] = sum_pairs (yT_scaled^T @ wo) accumulated IN PSUM (K=128 per
        pair step), single DVE bias add per m-chunk, immediate DMA out.

The softmax exp stream on the Scalar engine (64 x [128,1024] ACTIVATEs)
is the pacing resource; the PE pair loop, DVE reductions/copies, and
GPSIMD broadcasts all fit inside its envelope. Input DMAs are split into
per-chunk transfers across the sync/gpsimd/vector queues so mlp1 starts
~2us in (HAM warms early); constants land staggered by first use.
"""

import math

import numpy as np

B, S, F = 8, 1024, 512
H, DK = 8, 64
HID = 2 * F
P = 128
NPAIR = H // 2

N_CORES = 8

DEBUG = False

_CACHED_NC = None


def _build_nc(repeat=1):
    from contextlib import ExitStack

    import concourse.mybir as mybir
    import concourse.tile as tile
    from concourse import bacc

    dt = mybir.dt
    f32 = dt.float32
    bf16 = dt.bfloat16

    SC = S // P      # 8 token chunks
    FC = F // P      # 4 feature chunks
    KC = HID // P    # 8 hidden chunks
    NS = S // 512    # 2 moving-dim slices

    nc = bacc.Bacc(
        "TRN2",
        target_bir_lowering=False,
        debug=False,
        num_devices=N_CORES,
    )

    q_d = nc.declare_dram_parameter("qT", [F, S], bf16, isOutput=False)
    v_d = nc.declare_dram_parameter("vT", [F, S], bf16, isOutput=False)
    w1_d = nc.declare_dram_parameter("w1", [F, HID], bf16, isOutput=False)
    w2_d = nc.declare_dram_parameter("w2", [HID, F], bf16, isOutput=False)
    wv_d = nc.declare_dram_parameter("wv", [F, F], bf16, isOutput=False)
    wo_d = nc.declare_dram_parameter("wo", [F, F], bf16, isOutput=False)
    b1_d = nc.declare_dram_parameter("b1r", [P, KC], f32, isOutput=False)
    b2_d = nc.declare_dram_parameter("b2r", [P, FC], f32, isOutput=False)
    bv_d = nc.declare_dram_parameter("bvb", [P, F], f32, isOutput=False)
    bo_d = nc.declare_dram_parameter("bob", [P, F], f32, isOutput=False)
    id_d = nc.declare_dram_parameter("ident", [P, P], f32, isOutput=False)
    out_d = nc.declare_dram_parameter("out", [S, F], f32, isOutput=True)
    dbg = None
    if DEBUG:
        dbg = dict(
            hT=nc.declare_dram_parameter("dbg_hT", [P, KC * S], bf16,
                                         isOutput=True),
            aw=nc.declare_dram_parameter("dbg_aw", [P, FC * S], bf16,
                                         isOutput=True),
            v=nc.declare_dram_parameter("dbg_v", [P, SC * F], bf16,
                                        isOutput=True),
            eA0=nc.declare_dram_parameter("dbg_eA0", [P, SC * S], bf16,
                                          isOutput=True),
            eB0=nc.declare_dram_parameter("dbg_eB0", [P, SC * S], bf16,
                                          isOutput=True),
            rs=nc.declare_dram_parameter("dbg_rs", [P, H * SC], f32,
                                         isOutput=True),
            ri=nc.declare_dram_parameter("dbg_ri", [P, H * SC], f32,
                                         isOutput=True),
            rb=nc.declare_dram_parameter("dbg_rb", [P, S], f32,
                                         isOutput=True),
            yT=nc.declare_dram_parameter("dbg_yT", [P, FC * S], bf16,
                                         isOutput=True),
        )

    with ExitStack() as ctx:
        tc = ctx.enter_context(tile.TileContext(nc))

        const = ctx.enter_context(tc.tile_pool(name="const", bufs=1))
        big = ctx.enter_context(tc.tile_pool(name="big", bufs=1))
        sh16 = ctx.enter_context(tc.tile_pool(name="sh16", bufs=3))
        rpool = ctx.enter_context(tc.tile_pool(name="rpool", bufs=1))
        opool = ctx.enter_context(tc.tile_pool(name="opool", bufs=1))
        flip = ctx.enter_context(tc.tile_pool(name="flip", bufs=1))

        ps_sc = ctx.enter_context(tc.tile_pool(name="ps_sc", bufs=2, space="PSUM"))
        ps_yt = ctx.enter_context(tc.tile_pool(name="ps_yt", bufs=2, space="PSUM"))
        ps512 = ctx.enter_context(tc.tile_pool(name="ps512", bufs=2, space="PSUM"))

        # ---- constants + inputs: fine-grained DMAs, 3 queues, staged by
        # first use so mlp1's first matmul can start ~2us in ----
        w1sb = const.tile([P, FC, HID], bf16)
        w2sb = const.tile([P, KC, F], bf16)
        wvsb = const.tile([P, FC, F], bf16)
        wosb = const.tile([P, FC, F], bf16)
        b1sb = const.tile([P, KC], f32)
        b2sb = const.tile([P, FC], f32)
        bvsb = const.tile([P, F], f32)
        bosb = const.tile([P, F], f32)
        idsb = const.tile([P, P], f32)

        qTsb = big.tile([P, FC, S], bf16, tag="qx")
        valTsb = sh16.tile([P, FC, S], bf16, tag="sh")

        qr = q_d.rearrange("(c p) s -> p c s", p=P)
        w1r = w1_d.rearrange("(c p) k -> p c k", p=P)

        nc.sync.dma_start(b1sb, b1_d[:, :])
        for c in range(FC):
            nc.sync.dma_start(w1sb[:, c, :], w1r[:, c, :])
            nc.gpsimd.dma_start(qTsb[:, c, :512], qr[:, c, :512])
        nc.gpsimd.dma_start(qTsb[:, :, 512:], qr[:, :, 512:])
        nc.scalar.dma_start(w2sb, w2_d.rearrange("(c p) f -> p c f", p=P))
        nc.sync.dma_start(valTsb, v_d.rearrange("(c p) s -> p c s", p=P))
        nc.gpsimd.dma_start(wvsb, wv_d.rearrange("(c p) f -> p c f", p=P))
        nc.gpsimd.dma_start(b2sb, b2_d[:, :])
        nc.scalar.dma_start(wosb, wo_d.rearrange("(c p) f -> p c f", p=P))
        nc.sync.dma_start(bvsb, bv_d[:, :])
        nc.sync.dma_start(bosb, bo_d[:, :])
        nc.gpsimd.dma_start(idsb, id_d[:, :])

        consts = (w1sb, w2sb, wvsb, wosb, b1sb, b2sb, bvsb, bosb, idsb)
        for _rep in range(repeat):
            _build_body(nc, mybir, big, sh16, rpool, opool, flip,
                        ps_sc, ps_yt, ps512, qTsb, valTsb, out_d, consts,
                        dbg)

    nc.compile()
    return nc


def _build_body(nc, mybir, big, sh16, rpool, opool, flip,
                ps_sc, ps_yt, ps512, qTsb, valTsb, out_d, consts, dbg=None):
    w1sb, w2sb, wvsb, wosb, b1sb, b2sb, bvsb, bosb, idsb = consts
    dt = mybir.dt
    AF = mybir.ActivationFunctionType
    ALU = mybir.AluOpType
    AX = mybir.AxisListType
    f32, bf16 = dt.float32, dt.bfloat16
    SC, FC, KC, NS = S // P, F // P, HID // P, S // 512

    # ---- mlp1: hT = relu(w1^T @ qT + b1)  [HID, S] ----
    hTsb = big.tile([P, KC, S], bf16, tag="hT")
    for n in range(NS):
        for m in range(KC):
            h_p = ps512.tile([P, 512], f32, tag="ps")
            for c in range(FC):
                nc.tensor.matmul(
                    h_p,
                    w1sb[:, c, m * P:(m + 1) * P],
                    qTsb[:, c, n * 512:(n + 1) * 512],
                    start=(c == 0),
                    stop=(c == FC - 1),
                )
            nc.vector.tensor_scalar(
                hTsb[:, m, n * 512:(n + 1) * 512], h_p,
                b1sb[:, m:m + 1], 0.0, ALU.add, ALU.max,
            )

    # ---- mlp2 (chunk f feeds head pair f) ----
    awTsb = big.tile([P, FC, S], bf16, tag="awT")

    def mlp2_items(m):
        items = []
        state = {}

        def mk(n, c):
            def go():
                if c == 0:
                    state[n] = ps512.tile([P, 512], f32, tag="ps", name="m2p")
                nc.tensor.matmul(
                    state[n],
                    w2sb[:, c, m * P:(m + 1) * P],
                    hTsb[:, c, n * 512:(n + 1) * 512],
                    start=(c == 0),
                    stop=(c == KC - 1),
                )
                if c == KC - 1:
                    nc.vector.tensor_scalar_add(
                        awTsb[:, m, n * 512:(n + 1) * 512], state[n],
                        b2sb[:, m:m + 1],
                    )
            return go

        for n in range(NS):
            for c in range(KC):
                items.append(mk(n, c))
        return items

    # ---- v projection ----
    vsb = big.tile([P, SC, F], bf16, tag="v")

    def vproj_items(m):
        state = {}

        def mk(c):
            def go():
                if c == 0:
                    state[0] = ps512.tile([P, 512], f32, tag="ps", name="vpp")
                nc.tensor.matmul(
                    state[0],
                    valTsb[:, c, m * P:(m + 1) * P],
                    wvsb[:, c, :],
                    start=(c == 0),
                    stop=(c == FC - 1),
                )
                if c == FC - 1:
                    nc.vector.tensor_add(vsb[:, m, :], state[0], bvsb)
            return go

        return [mk(c) for c in range(FC)]

    # ---- per-pair state ----
    yTsb = big.tile([P, FC, S], bf16, tag="qx")  # reuses qT slot
    scale = 1.0 / math.sqrt(DK)
    rsum_all = rpool.tile([P, H, SC], f32, tag="rs")
    rinv_all = rpool.tile([P, H, SC], f32, tag="ri")
    o_all = opool.tile([P, SC, F], f32, tag="o")


    def pair_slot(pr, filler):
        """Head pair (2pr, 2pr+1): concurrent row-split scores, exp on ACT,
        DVE row sums, col-split attn_v interleaved at 1-chunk lag, then the
        rinv flip (PE transpose -> flatten DMA -> partition_broadcast) and
        the rinv-scaled yT copies. `filler` items (~1 matmul each) pad the
        PE stream while ACT chews the exps."""
        hA, hB = 2 * pr, 2 * pr + 1
        awp = awTsb[:, pr, :]
        eA = sh16.tile([P, SC, S], bf16, tag="sh")
        eB = sh16.tile([P, SC, S], bf16, tag="sh")

        yt_state = {}

        def attn_items(c):
            def mk(n, half, ldwf):
                def go():
                    if c == 0 and half == 0:
                        yt_state[n] = ps_yt.tile([P, 512], f32, tag="pt",
                                                 name="ytp")
                    e = eA if half == 0 else eB
                    h = hA if half == 0 else hB
                    out = (yt_state[n][0:DK, :] if half == 0
                           else yt_state[n][DK:P, :])
                    mm = nc.tensor.matmul(
                        out,
                        vsb[:, c, h * DK:(h + 1) * DK],
                        e[:, c, n * 512:(n + 1) * 512],
                        start=(c == 0),
                        stop=(c == SC - 1),
                        skip_group_check=True,
                    )
                    if ldwf:
                        mm.ins.ldweights = False
                return go

            return [mk(0, 0, False), mk(0, 1, False),
                    mk(1, 0, True), mk(1, 1, True)]

        per_slot = (len(filler) + SC - 1) // SC if filler else 0
        fi = 0
        for m in range(SC):
            pA = ps_sc.tile([P, S], f32, tag="sc")
            pB = ps_sc.tile([P, S], f32, tag="sc")
            for n in range(NS):
                mmA = nc.tensor.matmul(
                    pA[:, n * 512:(n + 1) * 512],
                    awp[0:DK, m * P:(m + 1) * P],
                    awp[0:DK, n * 512:(n + 1) * 512],
                    start=True, stop=True,
                )
                mmB = nc.tensor.matmul(
                    pB[:, n * 512:(n + 1) * 512],
                    awp[DK:P, m * P:(m + 1) * P],
                    awp[DK:P, n * 512:(n + 1) * 512],
                    start=True, stop=True,
                )
                if n == 1:
                    mmA.ins.ldweights = False
                    mmB.ins.ldweights = False
            nc.scalar.activation(eA[:, m, :], pA, AF.Exp, scale=scale,
                                 accum_out=rsum_all[:, hA, m:m + 1])
            nc.scalar.activation(eB[:, m, :], pB, AF.Exp, scale=scale,
                                 accum_out=rsum_all[:, hB, m:m + 1])
            if m >= 1:
                for go in attn_items(m - 1):
                    go()
            for _ in range(per_slot):
                if fi < len(filler):
                    filler[fi]()
                    fi += 1
        while fi < len(filler):
            filler[fi]()
            fi += 1
        for go in attn_items(SC - 1):
            go()

        nc.vector.reciprocal(rinv_all[:, hA:hA + 2, :],
                             rsum_all[:, hA:hA + 2, :])
        for n in range(NS):
            nc.vector.tensor_copy(
                yTsb[:, pr, n * 512:(n + 1) * 512], yt_state[n],
            )
        return eA, eB

    # ---- final: row-split pair matmuls (0,0)/(64,0), softmax division
    # fused into the per-head scalar_tensor_tensor accumulation ----
    def final_pair_items(pr):
        hA, hB = 2 * pr, 2 * pr + 1

        def mk(m):
            def go():
                oA = ps512.tile([P, 512], f32, tag="ps", name="fpA")
                nc.tensor.matmul(
                    oA, yTsb[0:DK, pr, m * P:(m + 1) * P],
                    wosb[0:DK, pr, :], start=True, stop=True,
                )
                oB = ps512.tile([P, 512], f32, tag="ps", name="fpB")
                nc.tensor.matmul(
                    oB, yTsb[DK:P, pr, m * P:(m + 1) * P],
                    wosb[DK:P, pr, :], start=True, stop=True,
                )
                nc.vector.scalar_tensor_tensor(
                    o_all[:, m, :], oA, rinv_all[:, hA, m:m + 1],
                    bosb if pr == 0 else o_all[:, m, :],
                    ALU.mult, ALU.add,
                )
                nc.vector.scalar_tensor_tensor(
                    o_all[:, m, :], oB, rinv_all[:, hB, m:m + 1],
                    o_all[:, m, :], ALU.mult, ALU.add,
                )
                if pr == NPAIR - 1:
                    nc.sync.dma_start(out_d[m * P:(m + 1) * P, :],
                                      o_all[:, m, :])
            return go

        return [mk(m) for m in range(SC)]

    # ---- software pipeline ----
    for go in mlp2_items(0):
        go()
    slot_fill = [
        (vproj_items(0) + vproj_items(1) + vproj_items(2) + vproj_items(3)
         + vproj_items(4) + vproj_items(5) + vproj_items(6) + vproj_items(7)
         + mlp2_items(1)),
        mlp2_items(2) + mlp2_items(3) + final_pair_items(0),
        final_pair_items(1),
        final_pair_items(2),
    ]
    e_keep = {}
    for pr in range(NPAIR):
        e_keep["eA"], e_keep["eB"] = pair_slot(pr, slot_fill[pr])
        if dbg is not None and pr == 0:
            nc.sync.dma_start(dbg["eA0"].rearrange("p (c s) -> p c s", c=SC),
                              e_keep["eA"])
            nc.sync.dma_start(dbg["eB0"].rearrange("p (c s) -> p c s", c=SC),
                              e_keep["eB"])
    if dbg is not None:
        nc.sync.dma_start(dbg["hT"].rearrange("p (c s) -> p c s", c=KC), hTsb)
        nc.sync.dma_start(dbg["aw"].rearrange("p (c s) -> p c s", c=FC),
                          awTsb)
        nc.sync.dma_start(dbg["v"].rearrange("p (c s) -> p c s", c=SC), vsb)
        nc.sync.dma_start(dbg["rs"].rearrange("p (h c) -> p h c", h=H),
                          rsum_all)
        nc.sync.dma_start(dbg["ri"].rearrange("p (h c) -> p h c", h=H),
                          rinv_all)
        nc.sync.dma_start(dbg["yT"].rearrange("p (c s) -> p c s", c=FC),
                          yTsb)

    # ---- last pair's final runs as the epilogue ----
    for go in final_pair_items(NPAIR - 1):
        go()


def _get_nc(repeat=1):
    global _CACHED_NC
    if _CACHED_NC is None:
        _CACHED_NC = _build_nc(repeat)
    return _CACHED_NC


def _make_in_maps(inputs):
    query = np.asarray(inputs["query"], np.float32)
    value = np.asarray(inputs["value"], np.float32)
    import ml_dtypes
    bf = ml_dtypes.bfloat16
    w1 = np.asarray(inputs["w1"], np.float32)
    b1 = np.asarray(inputs["b1"], np.float32)
    w2 = np.asarray(inputs["w2"], np.float32)
    b2 = np.asarray(inputs["b2"], np.float32)
    wv = np.asarray(inputs["wv"], np.float32)
    bv = np.asarray(inputs["bv"], np.float32)
    wo = np.asarray(inputs["wo"], np.float32)
    bo = np.asarray(inputs["bo"], np.float32)

    b1r = np.ascontiguousarray(b1.reshape(HID // P, P).T)
    b2r = np.ascontiguousarray(b2.reshape(F // P, P).T)
    bvb = np.ascontiguousarray(np.broadcast_to(bv, (P, F)))
    bob = np.ascontiguousarray(np.broadcast_to(bo, (P, F)))

    shared = dict(w1=w1.astype(bf), w2=w2.astype(bf), wv=wv.astype(bf),
                  wo=wo.astype(bf), b1r=b1r, b2r=b2r, bvb=bvb, bob=bob,
                  ident=np.eye(P, dtype=np.float32))
    return [dict(qT=np.ascontiguousarray(query[i].T).astype(bf),
                 vT=np.ascontiguousarray(value[i].T).astype(bf), **shared)
            for i in range(N_CORES)]


def kernel(**inputs):
    in_maps = _make_in_maps(inputs)

    from concourse.bass_utils import run_bass_kernel_spmd

    nc = _get_nc()
    res = run_bass_kernel_spmd(nc, in_maps, core_ids=list(range(N_CORES)))
    out = np.stack([res.results[i]["out"] for i in range(N_CORES)], axis=0)
    return out.astype(np.float32)


if __name__ == "__main__":
    nc = _get_nc()
    print("built ok")
